# revision 1
# baseline (speedup 1.0000x reference)
"""Tensor-parallel multi-head attention for Trainium2 (8 NeuronCores).

Problem: B=2, T=2048, E=1024, H=16 heads of dim 64.
  q/k/v = einsum('hei,bte->hbti'); s = q@k^T/sqrt(T); p = softmax(s)
  att = p@v; out = concat_heads(att) @ Wo^T

Sharding: tensor-parallel over heads — 2 heads per core. Each core computes
its heads' attention plus its slice of the output projection (Wo sharded
along its input axis); partial outputs are summed across cores.

Numerics: attention logits have std ~181 (unscaled randn weights), so softmax
is nearly one-hot and the QK^T path needs ~fp32 precision. We use an exact
hi/lo bf16 split (x = hi + lo, dropping only the lo*lo term) for the Q/K
projections and QK^T: 3 bf16 matmuls instead of 1 fp32 matmul (which costs
4x on the PE). Validated: adds ~3.5e-4 relative error. V path / PV / Wo run
in plain bf16 (~3.4e-3 total relative error).
"""

import sys

sys.path.insert(0, "/opt/trn_rl_repo")

import numpy as np
import ml_dtypes

import concourse.bass as bass
import concourse.mybir as mybir
import concourse.tile as tile
from concourse import bacc

BF16 = ml_dtypes.bfloat16
NF16 = np.float16

B, T, E = 2, 2048, 1024
H, I = 16, 64
NCORES = 8
HPC = H // NCORES            # heads per core = 2
BT = B * T                   # 4096
HI = HPC * I                 # 128 = per-core slice of the h*i axis
EC = E // 128                # 8 e-chunks
SCALE = 1.0 / float(np.sqrt(np.float32(T)))

F32 = mybir.dt.float32
BF = mybir.dt.bfloat16
FP16 = mybir.dt.float16

USE_HILO = True


def build_program(use_hilo: bool = USE_HILO, repeat: int = 1, phase_limit: int = 3) -> bass.Bass:
    nc = bacc.Bacc("TRN2", target_bir_lowering=False, debug=False,
                   num_devices=NCORES)

    # --- DRAM I/O (per-core contents supplied via in_maps) ---
    if use_hilo:
        xh_d = nc.dram_tensor("xh", [E, BT], FP16, kind="ExternalInput")
        xl_d = nc.dram_tensor("xl", [E, BT], FP16, kind="ExternalInput")
        wqh_d = nc.dram_tensor("wqh", [128, EC, HI], FP16, kind="ExternalInput")
        wql_d = nc.dram_tensor("wql", [128, EC, HI], FP16, kind="ExternalInput")
        wkh_d = nc.dram_tensor("wkh", [128, EC, HI], FP16, kind="ExternalInput")
        wkl_d = nc.dram_tensor("wkl", [128, EC, HI], FP16, kind="ExternalInput")
    else:
        xt_d = nc.dram_tensor("xt", [E, BT], F32, kind="ExternalInput")
        xh_d = nc.dram_tensor("xh", [E, BT], FP16, kind="ExternalInput")
        wq_d = nc.dram_tensor("wq", [128, EC, HI], F32, kind="ExternalInput")
        wk_d = nc.dram_tensor("wk", [128, EC, HI], F32, kind="ExternalInput")
    wv_d = nc.dram_tensor("wv", [128, EC, HI], FP16, kind="ExternalInput")
    wo_d = nc.dram_tensor("wo_t", [HI, E], FP16, kind="ExternalInput")
    out_d = nc.dram_tensor("out", [BT, E], FP16, kind="ExternalOutput")

    with tile.TileContext(nc) as tc:
        with (
            tc.tile_pool(name="psum", bufs=8, space="PSUM") as psp,
            tc.tile_pool(name="xstream", bufs=4) as xp,
            tc.tile_pool(name="weights", bufs=1) as wp,
            tc.tile_pool(name="persist", bufs=1) as pk,
            tc.tile_pool(name="big", bufs=1) as bigp,
            tc.tile_pool(name="ptile", bufs=3) as ptp,
            tc.tile_pool(name="stats", bufs=8) as stp,
            tc.tile_pool(name="evac", bufs=3) as evp,
        ):
            # --- load weights into SBUF ---
            if use_hilo:
                wqh = wp.tile([128, EC, HI], FP16, tag="wqh")
                wql = wp.tile([128, EC, HI], FP16, tag="wql")
                wkh = wp.tile([128, EC, HI], FP16, tag="wkh")
                wkl = wp.tile([128, EC, HI], FP16, tag="wkl")
                nc.sync.dma_start(wqh[:], wqh_d[:])
                nc.sync.dma_start(wql[:], wql_d[:])
                nc.sync.dma_start(wkh[:], wkh_d[:])
                nc.sync.dma_start(wkl[:], wkl_d[:])
            else:
                wq = wp.tile([128, EC, HI], F32, tag="wq")
                wk = wp.tile([128, EC, HI], F32, tag="wk")
                nc.sync.dma_start(wq[:], wq_d[:])
                nc.sync.dma_start(wk[:], wk_d[:])
            wv = wp.tile([128, EC, HI], FP16, tag="wv")
            nc.sync.dma_start(wv[:], wv_d[:])
            wo = wp.tile([128, E], FP16, tag="wo")
            nc.sync.dma_start(wo[:], wo_d[:])

            for _rep in range(repeat):
                # --- persistent activations ---
                if use_hilo:
                    Qh = pk.tile([128, BT], FP16, tag="Qh")
                    Ql = pk.tile([128, BT], FP16, tag="Ql")
                    K16 = pk.tile([128, BT], FP16, tag="K16")
                else:
                    QT = pk.tile([128, BT], F32, tag="QT")
                    KT = pk.tile([128, BT], F32, tag="KT")
                V = pk.tile([128, BT // 128, HI], FP16, tag="V")
                OT = pk.tile([128, BT], FP16, tag="OT")

                # ================= Phase 1: QKV projections =================
                # Q^T[i, t] = sum_e W[e, i] * xT[e, t]; t-banks of 512.
                for tb8 in range(BT // 512):
                    ts = slice(tb8 * 512, (tb8 + 1) * 512)
                    qt_ps = psp.tile([128, 512], F32, tag="ps")
                    kt_ps = psp.tile([128, 512], F32, tag="ps")
                    vt_ps = psp.tile([128, 512], F32, tag="ps")
                    if use_hilo:
                        # batched stream: 4 e-chunks per DMA
                        xbh = [None, None]
                        xbl = [None, None]
                        for h4 in range(2):
                            sl4 = slice(h4 * 512, (h4 + 1) * 512)
                            xbh[h4] = xp.tile([128, 4, 512], FP16, tag="xbh",
                                              name=f"xbh_{h4}")
                            nc.gpsimd.dma_start(
                                xbh[h4][:],
                                xh_d[sl4, ts].rearrange("(o p) t -> p o t",
                                                        p=128))
                            xbl[h4] = xp.tile([128, 4, 512], FP16, tag="xbl",
                                              name=f"xbl_{h4}")
                            nc.gpsimd.dma_start(
                                xbl[h4][:],
                                xl_d[sl4, ts].rearrange("(o p) t -> p o t",
                                                        p=128))
                    for ec in range(EC):
                        es = slice(ec * 128, (ec + 1) * 128)
                        if use_hilo:
                            xhs = xbh[ec // 4][:, ec % 4, :]
                            xls = xbl[ec // 4][:, ec % 4, :]
                            # Q^T += Wh'xh + Wh'xl + Wl'xh  (and same for K)
                            nc.tensor.matmul(qt_ps[:], wqh[:, ec, :], xhs[:],
                                             start=(ec == 0), stop=False)
                            nc.tensor.matmul(qt_ps[:], wqh[:, ec, :], xls[:],
                                             start=False, stop=False)
                            nc.tensor.matmul(qt_ps[:], wql[:, ec, :], xhs[:],
                                             start=False, stop=(ec == EC - 1))
                            nc.tensor.matmul(kt_ps[:], wkh[:, ec, :], xhs[:],
                                             start=(ec == 0), stop=False)
                            nc.tensor.matmul(kt_ps[:], wkh[:, ec, :], xls[:],
                                             start=False, stop=False)
                            nc.tensor.matmul(kt_ps[:], wkl[:, ec, :], xhs[:],
                                             start=False, stop=(ec == EC - 1))
                        else:
                            xfs = xp.tile([128, 512], F32, tag="xfs")
                            xhs = xp.tile([128, 512], BF, tag="xhs")
                            nc.gpsimd.dma_start(xfs[:], xt_d[es, ts])
                            nc.gpsimd.dma_start(xhs[:], xh_d[es, ts])
                            nc.tensor.matmul(qt_ps[:], wq[:, ec, :], xfs[:],
                                             start=(ec == 0), stop=(ec == EC - 1))
                            nc.tensor.matmul(kt_ps[:], wk[:, ec, :], xfs[:],
                                             start=(ec == 0), stop=(ec == EC - 1))
                        nc.tensor.matmul(vt_ps[:], wv[:, ec, :], xhs[:],
                                         start=(ec == 0), stop=(ec == EC - 1))

                    # evacuate; Q/K via hi/lo split (exact residual), V^T -> V
                    if use_hilo:
                        nc.scalar.copy(Qh[:, ts], qt_ps[:])
                        nc.vector.tensor_tensor(Ql[:, ts], qt_ps[:], Qh[:, ts],
                                                mybir.AluOpType.subtract)
                        nc.scalar.copy(K16[:, ts], kt_ps[:])
                    else:
                        nc.scalar.copy(QT[:, ts], qt_ps[:])
                        nc.scalar.copy(KT[:, ts], kt_ps[:])
                    vt_sb = evp.tile([128, 512], FP16, tag="vt")
                    nc.vector.tensor_copy(vt_sb[:], vt_ps[:])
                    # V^T slice [i=128, t=512] -> V[t-inner=128, 4 chunks, i=128]
                    nc.sync.dma_start_transpose(V[:, tb8 * 4:(tb8 + 1) * 4, :],
                                                vt_sb[:])

                # ================= Phase 2: attention per (b, head) =================
                def emit_pv(b, hr, PT):
                    # PV: O^T[i, t-bank] = sum_s V[s, i] * P^T[s, t]
                    for nb in range(4):
                        o_ps = psp.tile([64, 512], F32, tag="ps")
                        for sc in range(T // 128):
                            nc.tensor.matmul(
                                o_ps[:], V[:, b * (T // 128) + sc, hr],
                                PT[:, sc, nb * 512:(nb + 1) * 512],
                                start=(sc == 0), stop=(sc == T // 128 - 1))
                        nc.vector.tensor_copy(
                            OT[hr, b * T + nb * 512: b * T + (nb + 1) * 512],
                            o_ps[:])

                pending_pv = None
                for b in range(B if phase_limit >= 2 else 0):
                    for hh in range(HPC):
                        hr = slice(hh * 64, (hh + 1) * 64)
                        PT = bigp.tile([128, T // 128, T], FP16, tag="PT")
                        for tb in range(T // 128):
                            if tb == 4 and pending_pv is not None:
                                emit_pv(*pending_pv)
                                pending_pv = None
                            tcols = slice(b * T + tb * 128, b * T + (tb + 1) * 128)
                            s_ps = [psp.tile([128, 512], F32, tag="ps",
                                             name=f"s_ps_{j}")
                                    for j in range(4)]
                            if use_hilo:
                                for pi, lh in enumerate((Qh, Ql)):
                                    for j in range(4):
                                        scols = slice(b * T + j * 512,
                                                      b * T + (j + 1) * 512)
                                        nc.tensor.matmul(
                                            s_ps[j][:], lh[hr, tcols],
                                            K16[hr, scols],
                                            start=(pi == 0), stop=(pi == 1))
                            else:
                                for j in range(4):
                                    scols = slice(b * T + j * 512,
                                                  b * T + (j + 1) * 512)
                                    nc.tensor.matmul(
                                        s_ps[j][:], QT[hr, tcols], KT[hr, scols],
                                        start=True, stop=True)
                            # softmax over the free (s) axis
                            m4 = stp.tile([128, 4], F32, tag="m4")
                            for j in range(4):
                                nc.vector.reduce_max(m4[:, j:j + 1], s_ps[j][:],
                                                     axis=mybir.AxisListType.X)
                            negb = stp.tile([128, 1], F32, tag="negb")
                            nc.vector.reduce_max(negb[:], m4[:],
                                                 axis=mybir.AxisListType.X,
                                                 negate=True)
                            nc.vector.tensor_scalar_mul(negb[:], negb[:], SCALE)
                            Pt = ptp.tile([128, T], FP16, tag="Pt")
                            d4 = stp.tile([128, 4], F32, tag="d4")
                            for j in range(4):
                                nc.scalar.activation(
                                    Pt[:, j * 512:(j + 1) * 512], s_ps[j][:],
                                    mybir.ActivationFunctionType.Exp,
                                    bias=negb[:], scale=SCALE,
                                    accum_out=d4[:, j:j + 1])
                            den = stp.tile([128, 1], F32, tag="den")
                            nc.vector.reduce_sum(den[:], d4[:],
                                                 axis=mybir.AxisListType.X)
                            rcp = stp.tile([128, 1], F32, tag="rcp")
                            nc.vector.reciprocal(rcp[:], den[:])
                            nc.vector.tensor_scalar_mul(Pt[:], Pt[:], rcp[:])
                            # P block [t=128, s=T] -> P^T[s-inner, s-chunk, t-cols]
                            nc.sync.dma_start_transpose(PT[:, :, tb * 128:(tb + 1) * 128],
                                                        Pt[:])

                        pending_pv = (b, hr, PT)
                if pending_pv is not None:
                    emit_pv(*pending_pv)
                    pending_pv = None

                # ================= Phase 3: output projection =================
                # out[t, e] = sum_i OT[i, t] * wo[i, e]
                for obp in range(BT // 256 if phase_limit >= 3 else 0):
                    o_sb = evp.tile([128, 2, E], FP16, tag="osb")
                    for oo in range(2):
                        ob = obp * 2 + oo
                        trows = slice(ob * 128, (ob + 1) * 128)
                        for eb in range(E // 512):
                            w_ps = psp.tile([128, 512], F32, tag="ps")
                            nc.tensor.matmul(w_ps[:], OT[:, trows],
                                             wo[:, eb * 512:(eb + 1) * 512],
                                             start=True, stop=True)
                            nc.vector.tensor_copy(
                                o_sb[:, oo, eb * 512:(eb + 1) * 512], w_ps[:])
                    nc.gpsimd.dma_start(
                        out_d[obp * 256:(obp + 1) * 256, :].rearrange(
                            "(o p) e -> p o e", p=128),
                        o_sb[:])
    nc.compile()
    return nc


def _split_bf16(a32: np.ndarray):
    hi = a32.astype(BF16)
    lo = (a32 - hi.astype(np.float32)).astype(BF16)
    return hi, lo


def _split_fp16(a32: np.ndarray):
    hi = a32.astype(NF16)
    lo = (a32 - hi.astype(np.float32)).astype(NF16)
    return hi, lo


def make_in_maps(x, Wq, Wk, Wv, Wo, use_hilo: bool = USE_HILO):
    """Build the 8 per-core input maps from the full inputs."""
    x = np.asarray(x, np.float32)
    Wq = np.asarray(Wq, np.float32)
    Wk = np.asarray(Wk, np.float32)
    Wv = np.asarray(Wv, np.float32)
    Wo = np.asarray(Wo, np.float32)

    xt = np.ascontiguousarray(x.reshape(BT, E).T)          # [E, BT]
    xth, xtl = _split_bf16(xt)
    xth16, xtl16 = _split_fp16(xt)
    in_maps = []
    for c in range(NCORES):
        hsl = slice(c * HPC, (c + 1) * HPC)
        # [E, HPC*I] -> [EC, 128, HI]
        def _pmaj(w):  # [E, HI] -> [128, EC, HI] (partition-major)
            return np.ascontiguousarray(
                w.reshape(EC, 128, HI).transpose(1, 0, 2))
        wq_c = _pmaj(np.concatenate(list(Wq[hsl]), axis=1))
        wk_c = _pmaj(np.concatenate(list(Wk[hsl]), axis=1))
        wv_c = _pmaj(np.concatenate(list(Wv[hsl]), axis=1))
        wo_c = np.ascontiguousarray(Wo[:, c * HI:(c + 1) * HI].T)  # [HI, E]
        if use_hilo:
            m = {
                "wv": wv_c.astype(NF16),
                "wo_t": wo_c.astype(NF16),
            }
            m["xh"], m["xl"] = xth16, xtl16
            m["wqh"], m["wql"] = _split_fp16(wq_c)
            m["wkh"], m["wkl"] = _split_fp16(wk_c)
        else:
            m = {
                "wv": wv_c.astype(BF16),
                "wo_t": wo_c.astype(BF16),
            }
            m["xt"] = xt
            m["xh"] = xth
            m["wq"] = wq_c
            m["wk"] = wk_c
        in_maps.append(m)
    return in_maps


_CACHED = {}


def _get_program(use_hilo: bool = USE_HILO) -> bass.Bass:
    if use_hilo not in _CACHED:
        _CACHED[use_hilo] = build_program(use_hilo)
    return _CACHED[use_hilo]


def kernel(**inputs) -> np.ndarray:
    from concourse.bass_utils import run_bass_kernel_spmd

    nc = _get_program()
    in_maps = make_in_maps(inputs["x"], inputs["Wq"], inputs["Wk"],
                           inputs["Wv"], inputs["Wo"])
    res = run_bass_kernel_spmd(nc, in_maps, core_ids=list(range(NCORES)))
    out = np.zeros((BT, E), np.float32)
    for c in range(NCORES):
        out += np.asarray(res.results[c]["out"], np.float32)
    return out.reshape(B, T, E)



# revision 31
# speedup vs baseline: 1.0642x; 1.0642x over previous
"""Tensor-parallel multi-head attention for Trainium2 (8 NeuronCores) — v2.

Problem: B=2, T=2048, E=1024, H=16 heads of dim 64.
  q/k/v = einsum('hei,bte->hbti'); s = q@k^T/sqrt(T); p = softmax(s)
  att = p@v; out = concat_heads(att) @ Wo^T

Sharding: tensor-parallel over heads — 2 heads per core (partitions 0-63 =
head A, 64-127 = head B throughout). Wo sharded along its input axis;
partial outputs summed on host in fp32.

v2 vs v1 (646us -> target ~170us):
  - All projections single-pass fp16 (validated numerically: rel err 7e-3
    vs 2e-2 budget; hi/lo splits dropped).
  - 1/sqrt(T) folded into Q at projection evacuation.
  - QK for the two heads runs on complementary PE row-halves (concurrent
    row-tiled matmuls), softmax over a contiguous 4-bank PSUM row.
  - P left unnormalized; normalization deferred to the PV-output evac via
    a reciprocal-denominator tile broadcast across partitions (DRAM bounce).
  - PV packs both heads as concurrent col-tiled matmuls (full PE width).
  - Wo interleaved per 512-token group; psum written straight to DRAM fp32.
"""

import sys

sys.path.insert(0, "/opt/trn_rl_repo")

import numpy as np
import ml_dtypes

import concourse.bass as bass
import concourse.mybir as mybir
import concourse.tile as tile
from concourse import bacc

NF16 = np.float16

B, T, E = 2, 2048, 1024
H, I = 16, 64
NCORES = 8
HPC = H // NCORES            # heads per core = 2
BT = B * T                   # 4096
HI = HPC * I                 # 128 = per-core slice of the h*i axis
EC = E // 128                # 8 e-chunks
NTB = T // 128               # 16 t-blocks per batch
SCALE = 1.0 / float(np.sqrt(np.float32(T)))

F32 = mybir.dt.float32
FP16 = mybir.dt.float16


def build_program(repeat: int = 1, debug_outputs: bool = False) -> bass.Bass:
    nc = bacc.Bacc("TRN2", target_bir_lowering=False, debug=False,
                   num_devices=NCORES)

    x_d = nc.dram_tensor("x16", [E, BT], FP16, kind="ExternalInput")
    wq_d = nc.dram_tensor("wq16", [128, EC, HI], FP16, kind="ExternalInput")
    wk_d = nc.dram_tensor("wk16", [128, EC, HI], FP16, kind="ExternalInput")
    wv_d = nc.dram_tensor("wv16", [128, EC, HI], FP16, kind="ExternalInput")
    wo_d = nc.dram_tensor("wo16", [HI, E], FP16, kind="ExternalInput")
    out_d = nc.dram_tensor("out", [BT, E], FP16, kind="ExternalOutput")
    # one-hot selector for the rcp row-broadcast matmuls: output partition
    # p of matmul tbl picks moving row tbl (p<64, head 0) or 4+tbl (head 1).
    e8_np = np.zeros((8, 4, 128), np.float16)
    for t_ in range(4):
        e8_np[t_, t_, 0:64] = 1.0
        e8_np[4 + t_, t_, 64:128] = 1.0
    e8_d = nc.inline_tensor(e8_np.reshape(8, 512), name="e8")
    if debug_outputs:
        dbgot_d = nc.dram_tensor("dbg_ot", [128, BT], FP16,
                                 kind="ExternalOutput")
        dbgr_d = nc.dram_tensor("dbg_r", [B, 4, 128, 512], FP16,
                                kind="ExternalOutput")
        dbgden_d = nc.dram_tensor("dbg_den", [128, B, HPC, NTB], F32,
                                  kind="ExternalOutput")
        dbgq_d = nc.dram_tensor("dbg_q", [128, BT], FP16,
                                kind="ExternalOutput")
        dbgk_d = nc.dram_tensor("dbg_k", [128, BT], FP16,
                                kind="ExternalOutput")
        dbgv_d = nc.dram_tensor("dbg_v", [128, BT // 128, HI], FP16,
                                kind="ExternalOutput")

    with tile.TileContext(nc) as tc:
        with (
            tc.tile_pool(name="psum", bufs=2, space="PSUM") as psp,
            tc.tile_pool(name="weights", bufs=1) as wp,
            tc.tile_pool(name="persist", bufs=1) as pk,
            tc.tile_pool(name="xstream", bufs=4) as xp,
            tc.tile_pool(name="ptile", bufs=3) as ptp,
            tc.tile_pool(name="pttile", bufs=6) as PTp,
            tc.tile_pool(name="stats", bufs=8) as stp,
            tc.tile_pool(name="evac", bufs=3) as evp,
            tc.tile_pool(name="rtile", bufs=2) as rp,
        ):
            wq = wp.tile([128, EC, HI], FP16, tag="wq")
            wk = wp.tile([128, EC, HI], FP16, tag="wk")
            wv = wp.tile([128, EC, HI], FP16, tag="wv")
            wo = wp.tile([128, E], FP16, tag="wo")
            e8 = wp.tile([8, 4, 128], FP16, tag="e8")
            nc.sync.dma_start(wq[:], wq_d[:])
            nc.sync.dma_start(wk[:], wk_d[:])
            nc.sync.dma_start(wv[:], wv_d[:])
            nc.sync.dma_start(wo[:], wo_d[:])
            nc.sync.dma_start(e8[:], e8_d[:].rearrange("k (t p) -> k t p", t=4))

            # fp16 rcp staging, padded to 128 cols for the xbar transpose;
            # cols 0:4 head 0, 4:8 head 1; pad columns zeroed once so the
            # transpose never reads uninit.
            rcp_pad = pk.tile([128, 128], FP16, tag="rcp_pad")
            nc.gpsimd.memset(rcp_pad[:], 0.0)

            for _rep in range(repeat):
                Q16 = pk.tile([128, BT], FP16, tag="Q16")
                K16 = pk.tile([128, BT], FP16, tag="K16")
                V = pk.tile([128, BT // 128, HI], FP16, tag="V")
                OT = pk.tile([128, BT], FP16, tag="OT")
                den = pk.tile([128, B, HPC, NTB], F32, tag="den")

                # ---------------- phase 1: QKV projections ----------------
                def proj_bank(tb8):
                    ts = slice(tb8 * 512, (tb8 + 1) * 512)
                    ps = psp.tile([128, 2048], F32, tag="S", name="ps_proj")
                    qt, kt, vt = ps[:, 0:512], ps[:, 512:1024], ps[:, 1024:1536]
                    xb = [None, None]
                    for h4 in range(2):
                        sl4 = slice(h4 * 512, (h4 + 1) * 512)
                        xb[h4] = xp.tile([128, 4, 512], FP16, tag="xb",
                                         name=f"xb_{h4}")
                        nc.gpsimd.dma_start(
                            xb[h4][:],
                            x_d[sl4, ts].rearrange("(o p) t -> p o t", p=128))
                    for ec in range(EC):
                        xs = xb[ec // 4][:, ec % 4, :]
                        nc.tensor.matmul(qt[:], wq[:, ec, :], xs[:],
                                         start=(ec == 0), stop=(ec == EC - 1))
                        nc.tensor.matmul(kt[:], wk[:, ec, :], xs[:],
                                         start=(ec == 0), stop=(ec == EC - 1))
                        nc.tensor.matmul(vt[:], wv[:, ec, :], xs[:],
                                         start=(ec == 0), stop=(ec == EC - 1))
                    nc.scalar.mul(Q16[:, ts], qt[:], SCALE)
                    nc.scalar.copy(K16[:, ts], kt[:])
                    vt_sb = evp.tile([128, 512], FP16, tag="vt")
                    nc.vector.tensor_copy(vt_sb[:], vt[:])
                    nc.sync.dma_start_transpose(V[:, tb8 * 4:(tb8 + 1) * 4, :],
                                                vt_sb[:])

                # -------------- phase 2 helpers --------------
                def qk_softmax(b, nb, tbl, PTh):
                    tb = nb * 4 + tbl
                    tcols = slice(b * T + tb * 128, b * T + (tb + 1) * 128)
                    ps2 = [psp.tile([128, 2048], F32, tag="S",
                                    name=f"ps_s{hh}") for hh in range(HPC)]
                    m4 = [stp.tile([128, 4], F32, tag="m4", name=f"m4_{hh}")
                          for hh in range(HPC)]
                    for hh in range(HPC):
                        hr = slice(hh * 64, (hh + 1) * 64)
                        for j in range(4):
                            scols = slice(b * T + j * 512,
                                          b * T + (j + 1) * 512)
                            nc.tensor.matmul(ps2[hh][:, j * 512:(j + 1) * 512],
                                             Q16[hr, tcols], K16[hr, scols],
                                             start=True, stop=True)
                    # reduces emitted after all 8 matmuls: a consumer that
                    # fires the instant a matmul's semaphore lands can read a
                    # bank whose drain hasn't finished; batching keeps >=4
                    # matmuls of slack between producer and reader.
                    for hh in range(HPC):
                        for j in range(4):
                            nc.vector.reduce_max(
                                m4[hh][:, j:j + 1],
                                ps2[hh][:, j * 512:(j + 1) * 512],
                                axis=mybir.AxisListType.X)
                    for hh in range(HPC):
                        negb = stp.tile([128, 1], F32, tag="negb")
                        nc.vector.reduce_max(negb[:], m4[hh][:],
                                             axis=mybir.AxisListType.X,
                                             negate=True)
                        Pt = ptp.tile([128, 2048], FP16, tag="Pt")
                        nc.scalar.activation(
                            Pt[:], ps2[hh][:],
                            mybir.ActivationFunctionType.Exp,
                            bias=negb[:], scale=1.0,
                            accum_out=den[:, b, hh, tb:tb + 1])
                        eng = nc.sync if (tb + hh) % 2 == 0 else nc.scalar
                        eng.dma_start_transpose(
                            PTh[hh][:, :, tbl * 128:(tbl + 1) * 128], Pt[:])

                def build_r(b, nb):
                    # R[p, 128*tbl + tt] = 1/den[tt, b, hh(p), nb*4+tbl]:
                    # reciprocal (both heads into pad cols 0:8) -> one xbar
                    # transpose (tb index to partitions) -> one selector
                    # matmul per tbl broadcasts the right row to each
                    # partition half, each into its own PSUM bank -> evac.
                    for hh in range(HPC):
                        with nc.allow_low_precision("rcp in fp16"):
                            nc.vector.reciprocal(
                                rcp_pad[:, hh * 4:hh * 4 + 4],
                                den[:, b, hh, nb * 4:(nb + 1) * 4])
                    rcpT = evp.tile([128, 128], FP16, tag="rcpT")
                    nc.sync.dma_start_transpose(rcpT[:], rcp_pad[:])
                    r_tile = psp.tile([128, 2048], F32, tag="S", name="ps_r")
                    for tbl in range(4):
                        nc.tensor.matmul(
                            r_tile[:, tbl * 512:tbl * 512 + 128],
                            e8[:, tbl, :], rcpT[0:8, 0:128],
                            start=True, stop=True)
                    return r_tile

                def pv_evac_wo(b, nb, PTh):
                    r_tile = build_r(b, nb)
                    # each head's accumulation chain gets its own PSUM bank:
                    # start=True clears the whole bank, so interleaved chains
                    # must not share one.
                    o_tile = psp.tile([128, 2048], F32, tag="S", name="ps_o")
                    for hh in range(HPC):
                        hr = slice(hh * 64, (hh + 1) * 64)
                        for sc in range(T // 128):
                            ch = b * (T // 128) + sc
                            nc.tensor.matmul(
                                o_tile[hr, hh * 512:(hh + 1) * 512],
                                V[:, ch, hr], PTh[hh][:, sc, :],
                                start=(sc == 0),
                                stop=(sc == T // 128 - 1))
                    # rt copy deferred to here: 32 PV matmuls of slack since
                    # the R-broadcast matmuls wrote r_tile (drain-race guard).
                    rt = rp.tile([128, 512], FP16, tag="R")
                    rv = r_tile.rearrange("p (a c) -> p a c", a=4)[:, :, 0:128]
                    nc.vector.tensor_copy(
                        rt[:].rearrange("p (a c) -> p a c", a=4), rv)
                    if debug_outputs:
                        nc.scalar.dma_start(dbgr_d[b, nb], rt[:])
                    # head-0 evac first: its chain finished 16 matmuls ago
                    # (drained); its 512-cycle read then covers head 1's drain.
                    ocols = slice(b * T + nb * 512, b * T + (nb + 1) * 512)
                    for hh in range(HPC):
                        hr = slice(hh * 64, (hh + 1) * 64)
                        nc.vector.tensor_tensor(
                            OT[hr, ocols], o_tile[hr, hh * 512:(hh + 1) * 512],
                            rt[hr, :], mybir.AluOpType.mult)
                    # Wo for this 512-token group: 4 row-blocks of 128
                    for half in range(2):
                        w_tile = psp.tile([128, 2048], F32, tag="S",
                                          name="ps_w")
                        for k in range(2):
                            tr = nb * 4 + half * 2 + k
                            trows = slice(b * T + tr * 128,
                                          b * T + (tr + 1) * 128)
                            for eb in range(2):
                                nc.tensor.matmul(
                                    w_tile[:, (k * 2 + eb) * 512:
                                           (k * 2 + eb + 1) * 512],
                                    OT[:, trows], wo[:, eb * 512:(eb + 1) * 512],
                                    start=True, stop=True)
                        o_sb = evp.tile([128, 2, E], FP16, tag="osb")
                        wv2 = w_tile.rearrange("p (o e) -> p o e", o=2)
                        if half == 0:
                            nc.vector.tensor_copy(o_sb[:], wv2[:])
                        else:
                            nc.scalar.copy(o_sb[:], wv2[:])
                        r0 = b * T + (nb * 4 + half * 2) * 128
                        nc.gpsimd.dma_start(
                            out_d[r0:r0 + 256, :].rearrange(
                                "(o p) e -> p o e", p=128),
                            o_sb[:])

                # -------------- emission --------------
                for tb8 in range(4):
                    proj_bank(tb8)

                pending = None
                for b in range(B):
                    for nb in range(4):
                        if b == 0:
                            proj_bank(4 + nb)
                        PTh = [PTp.tile([128, NTB, 512], FP16, tag="PT",
                                        name=f"PT_{b}_{nb}_{hh}")
                               for hh in range(HPC)]
                        for tbl in range(4):
                            qk_softmax(b, nb, tbl, PTh)
                        if pending is not None:
                            pv_evac_wo(*pending)
                        pending = (b, nb, PTh)
                if pending is not None:
                    pv_evac_wo(*pending)
                if debug_outputs:
                    nc.scalar.dma_start(dbgot_d[:], OT[:])
                    nc.scalar.dma_start(dbgden_d[:], den[:])
                    nc.scalar.dma_start(dbgq_d[:], Q16[:])
                    nc.scalar.dma_start(dbgk_d[:], K16[:])
                    nc.scalar.dma_start(dbgv_d[:], V[:])
    nc.compile()
    return nc


def make_in_maps(x, Wq, Wk, Wv, Wo):
    """Build the 8 per-core input maps from the full inputs."""
    x = np.asarray(x, np.float32)
    Wq = np.asarray(Wq, np.float32)
    Wk = np.asarray(Wk, np.float32)
    Wv = np.asarray(Wv, np.float32)
    Wo = np.asarray(Wo, np.float32)

    xt = np.ascontiguousarray(x.reshape(BT, E).T).astype(NF16)   # [E, BT]
    in_maps = []
    for c in range(NCORES):
        hsl = slice(c * HPC, (c + 1) * HPC)

        def _pmaj(w):  # [E, HI] -> [128, EC, HI] (partition-major)
            return np.ascontiguousarray(
                w.reshape(EC, 128, HI).transpose(1, 0, 2)).astype(NF16)

        m = {
            "x16": xt,
            "wq16": _pmaj(np.concatenate(list(Wq[hsl]), axis=1)),
            "wk16": _pmaj(np.concatenate(list(Wk[hsl]), axis=1)),
            "wv16": _pmaj(np.concatenate(list(Wv[hsl]), axis=1)),
            "wo16": np.ascontiguousarray(
                Wo[:, c * HI:(c + 1) * HI].T).astype(NF16),
        }
        in_maps.append(m)
    return in_maps


_CACHED = {}


def _get_program() -> bass.Bass:
    if "nc" not in _CACHED:
        _CACHED["nc"] = build_program()
    return _CACHED["nc"]


def kernel(**inputs) -> np.ndarray:
    from concourse.bass_utils import run_bass_kernel_spmd

    nc = _get_program()
    in_maps = make_in_maps(inputs["x"], inputs["Wq"], inputs["Wk"],
                           inputs["Wv"], inputs["Wo"])
    res = run_bass_kernel_spmd(nc, in_maps, core_ids=list(range(NCORES)))
    out = np.zeros((BT, E), np.float32)
    for c in range(NCORES):
        out += np.asarray(res.results[c]["out"], np.float32)
    return out.reshape(B, T, E)
